# revision 36
# baseline (speedup 1.0000x reference)
"""Trainium2 Bass kernel for nn_Attention_Module (dense_transformer).

Data-parallel over batch: B=64 split across 8 NeuronCores (8 per core).
Per core, all activations are kept channel-major [C, tokens] with the
8 local batches' 320 tokens reordered into a z-block (8*64=512 template
tokens) followed by an x-block (8*256=2048 search tokens), processed as
5 token-tiles of 512.

Dense matmuls run in fp8(e4m3) DoubleRow perf mode (two 128-row k-tiles
per instruction, PSUM accumulation fp32); the bf16 residual spine
(r = y1+u1, t1, pre, LN) keeps the output under the accuracy gate.
1/sqrt is computed as exp(-0.5*ln(x)) and the activation-table chooser
is pinned to the single table serving ln/exp/relu/identity/copy, so
exactly one ACT_TABLE_LOAD is emitted.

All data reordering (token regrouping, weight/bias packing, fp8 casts,
output unpermute) happens host-side so every device DMA is a contiguous
partition-line transfer.

Self-contained: only imports infra from /opt/trn_rl_repo.
"""
import sys

sys.path.insert(0, "/opt/trn_rl_repo")

from contextlib import ExitStack

import numpy as np

import os

import concourse.bacc as bacc
import concourse.tile as tile
from concourse import mybir

KV_GPS = os.environ.get("KV_GPS", "1") == "1"    # gpsimd residual add
KV_EXPB = os.environ.get("KV_EXPB", "1") == "1"  # batched strided exp acts
KV_ID = os.environ.get("KV_ID", "1") == "1"      # Identity act for LN apply
F32 = mybir.dt.float32
BF = mybir.dt.bfloat16
F8 = mybir.dt.float8e4
AF = mybir.ActivationFunctionType
OP = mybir.AluOpType
AX = mybir.AxisListType
DR = mybir.MatmulPerfMode.DoubleRow

B_LOC = 8          # batches per core
DIM = 512
HID = 256
HEADS = 8
NZ, NX = 64, 256   # template / search tokens per batch
NTOK = NZ + NX     # 320
NT = 5             # token tiles of 512
EPS_LN = 1e-5
TINY = 1e-24       # guards ln/rsqrt of exact-zero row norms

# packed fp8 weight blob column offsets
OFF_WLINU = 0          # [4, 512]  W_lin u-half (h cols 512:1024)
OFF_WLINY = 2048       # [4, 512]  W_lin y-half (h cols 0:512)
OFF_WDOWN = 4096       # [4, 256]
OFF_WQ = 5120          # [2, 256]  (WqT)
OFF_WK = 5632
OFF_WV = 6144
OFF_WO = 6656
OFF_WUP = 7168         # [2, 512]
OFF_WEND = 8192        # [4, 512]
W_COLS = 10240


def _bbs(j):
    """Branch segments inside token-tile j: list of (batch, col_off, width).

    Tile 0 is the z-block (8 branches of 64), tiles 1..4 hold two x-branches
    of 256 tokens each (batches 2j-2 and 2j-1).
    """
    if j == 0:
        return [(b, 64 * b, 64) for b in range(B_LOC)]
    return [(2 * (j - 1), 0, 256), (2 * j - 1, 256, 256)]


def _pin_act_table():
    """Pin the activation-table chooser to natural_log_exp_and_others.

    All activation functions this kernel uses (relu/ln/exp/identity/copy)
    genuinely live in that one table, but the automatic chooser ping-pongs
    between the ln-only and exp-only tables, inserting ~30 ACT_TABLE_LOADs
    (1.28us each).  Claiming the other tables serve nothing makes the
    fixpoint pass emit a single load of the correct table.  Indices stay
    canonical (full dict, original order), so the emitted act_func_set_id
    is still valid for walrus.
    """
    import functools
    import concourse.hw_specs as hw_specs

    orig = hw_specs.get_activation_tables.__wrapped__

    @functools.cache
    def patched(arch):
        t = orig(arch)
        keep = "natural_log_exp_and_others"
        return {k: (v if k == keep else set()) for k, v in t.items()}

    bacc.get_activation_tables = patched


def build_nc():
    _pin_act_table()
    nc = bacc.Bacc("TRN2", target_bir_lowering=False, debug=False,
                   num_devices=8)

    # ---- DRAM I/O (per-core shapes, all contiguous partition lines) ----
    x18_e = nc.declare_dram_parameter("x1f8", [NT, 128, 4, 512], F8, isOutput=False)
    x28_e = nc.declare_dram_parameter("x2f8", [NT, 128, 4, 512], F8, isOutput=False)
    x1b_e = nc.declare_dram_parameter("x1bf", [NT, 128, 4, 512], BF, isOutput=False)
    out_e = nc.declare_dram_parameter("out", [NT, 128, 4, 512], BF, isOutput=True)
    wall_e = nc.declare_dram_parameter("wall8", [128, W_COLS], F8, isOutput=False)
    ball_e = nc.declare_dram_parameter("ball", [128, 28], F32, isOutput=False)
    ones_e = nc.declare_dram_parameter("ones_in", [128, 128], BF, isOutput=False)
    ident_e = nc.declare_dram_parameter("ident_in", [128, 128], BF, isOutput=False)
    zeros_e = nc.declare_dram_parameter("zeros_in", [128, 2, 8, 128], BF, isOutput=False)
    zeros2_e = nc.declare_dram_parameter("zeros2_in", [128, 8, 2, 128], BF, isOutput=False)

    with tile.TileContext(nc) as tc, ExitStack() as ctx:
        wts = ctx.enter_context(tc.tile_pool(name="wts", bufs=1))
        xload = ctx.enter_context(tc.tile_pool(name="xload", bufs=3))
        u1p = ctx.enter_context(tc.tile_pool(name="u1p", bufs=2))
        rp = ctx.enter_context(tc.tile_pool(name="rp", bufs=2))
        u2p = ctx.enter_context(tc.tile_pool(name="u2p", bufs=2))
        ap_ = ctx.enter_context(tc.tile_pool(name="ap", bufs=2))
        bqp = ctx.enter_context(tc.tile_pool(name="bqp", bufs=2))
        qkvp = ctx.enter_context(tc.tile_pool(name="qkvp", bufs=2))
        sqp = ctx.enter_context(tc.tile_pool(name="sqp", bufs=2))
        nrmp = ctx.enter_context(tc.tile_pool(name="nrmp", bufs=2))
        qtp = ctx.enter_context(tc.tile_pool(name="qtp", bufs=2))
        ep = ctx.enter_context(tc.tile_pool(name="ep", bufs=2))
        avp = ctx.enter_context(tc.tile_pool(name="avp", bufs=2))
        o1p = ctx.enter_context(tc.tile_pool(name="o1p", bufs=2))
        scr = ctx.enter_context(tc.tile_pool(name="scr", bufs=2))
        yp = ctx.enter_context(tc.tile_pool(name="yp", bufs=2))
        prep = ctx.enter_context(tc.tile_pool(name="prep", bufs=2))
        statp = ctx.enter_context(tc.tile_pool(name="statp", bufs=2))
        outp = ctx.enter_context(tc.tile_pool(name="outp", bufs=2))
        # PSUM: 8 banks of [128, 512] fp32:
        #   ps2 = paired 2-bank tiles (2 bufs), ps = single (2), atp = attn (2)
        ps = ctx.enter_context(tc.tile_pool(name="ps", bufs=2, space="PSUM"))
        ps2 = ctx.enter_context(tc.tile_pool(name="ps2", bufs=2, space="PSUM"))
        atp = ctx.enter_context(tc.tile_pool(name="atp", bufs=2, space="PSUM"))

        # ---- weights / constants to SBUF ----
        wall_sb = wts.tile([128, W_COLS], F8)
        wlinU_sb = wall_sb[:, OFF_WLINU:OFF_WLINU + 2048].rearrange(
            "p (kt m) -> p kt m", kt=4)
        wlinY_sb = wall_sb[:, OFF_WLINY:OFF_WLINY + 2048].rearrange(
            "p (kt m) -> p kt m", kt=4)
        wdown_sb = wall_sb[:, OFF_WDOWN:OFF_WDOWN + 1024].rearrange(
            "p (kt m) -> p kt m", kt=4)
        wq_sb = wall_sb[:, OFF_WQ:OFF_WQ + 512].rearrange(
            "p (kt m) -> p kt m", kt=2)
        wk_sb = wall_sb[:, OFF_WK:OFF_WK + 512].rearrange(
            "p (kt m) -> p kt m", kt=2)
        wv_sb = wall_sb[:, OFF_WV:OFF_WV + 512].rearrange(
            "p (kt m) -> p kt m", kt=2)
        wo_sb = wall_sb[:, OFF_WO:OFF_WO + 512].rearrange(
            "p (kt m) -> p kt m", kt=2)
        wup_sb = wall_sb[:, OFF_WUP:OFF_WUP + 1024].rearrange(
            "p (kt m) -> p kt m", kt=2)
        wend_sb = wall_sb[:, OFF_WEND:OFF_WEND + 2048].rearrange(
            "p (kt m) -> p kt m", kt=4)

        ball_sb = wts.tile([128, 28], F32)
        blin_sb = ball_sb[:, 0:8]
        bdown_sb = ball_sb[:, 8:10]
        bup_sb = ball_sb[:, 10:14]
        bend_sb = ball_sb[:, 14:18]
        gamma_sb = ball_sb[:, 18:22]
        beta_sb = ball_sb[:, 22:26]
        tempc_sb = ball_sb[:, 26:28]
        lntemp_sb = wts.tile([128, 2], F32)

        ones_sb = wts.tile([128, 128], BF)
        ident_sb = wts.tile([128, 128], BF)
        bd = wts.tile([128, 2, 8, 128], BF)
        # zero-padded per-branch kT for the z-tile: avoids 64-row sub-tile
        # matmuls at alternating partition bases (bf16 PE faults on those)
        ktz = wts.tile([128, 8, 2, 128], BF)
        tiny_sb = wts.tile([128, 1], F32)
        nc.vector.memset(tiny_sb[:], TINY)
        epsln_sb = wts.tile([128, 1], F32)
        nc.vector.memset(epsln_sb[:], EPS_LN)

        def emit_loads(j, hook=None):
            # ---- S1: load X (host pre-reordered, contiguous); fp8 feeds
            # the matmuls, a bf16 copy of x1 is the LN residual.  `hook`
            # (first tile only) issues the critical weight DMAs right after
            # the kp=0 chunks so the first matmul isn't queued behind the
            # full x load ----
            x1t = xload.tile([128, 4, 512], F8, tag="xl")
            x2t = xload.tile([128, 4, 512], F8, tag="xl")
            x1b = xload.tile([128, 4, 512], BF, tag="xb")
            for kp in range(2):
                nc.sync.dma_start(x1t[:, 2 * kp:2 * kp + 2, :],
                                  x18_e[j, :, 2 * kp:2 * kp + 2, :])
                nc.sync.dma_start(x2t[:, 2 * kp:2 * kp + 2, :],
                                  x28_e[j, :, 2 * kp:2 * kp + 2, :])
                if kp == 0 and hook is not None:
                    hook()
            nc.sync.dma_start(x1b[:], x1b_e[j, :, :, :])
            return (x1t, x2t, x1b)

        def emit_front_S1u(j, ld):
            x1t, x2t, x1b = ld
            # ---- S1: u-halves of h1/h2 (fp8 DoubleRow); u1/u2 share a
            # 2-bank PSUM pair so one relu act covers both ----
            u12 = u1p.tile([128, 2, 4, 512], F8)
            for m in range(4):
                pt12 = ps2.tile([128, 2, 512], F32, tag="ps2")
                for kp in range(2):
                    w = wlinU_sb[:, 2 * kp:2 * kp + 2, 128 * m:128 * (m + 1)]
                    nc.tensor.matmul(pt12[:, 0, :], w, x1t[:, 2 * kp:2 * kp + 2, :],
                                     perf_mode=DR,
                                     start=(kp == 0), stop=(kp == 1))
                    nc.tensor.matmul(pt12[:, 1, :], w, x2t[:, 2 * kp:2 * kp + 2, :],
                                     perf_mode=DR,
                                     start=(kp == 0), stop=(kp == 1))
                nc.scalar.activation(u12[:, :, m, :], pt12[:], AF.Relu,
                                     bias=blin_sb[:, m + 4:m + 5])
            return u12

        def emit_front_rest(j, ld, u12):
            x1t, x2t, x1b = ld
            u1 = u12[:, 0]
            u2 = u12[:, 1]
            r = rp.tile([128, 4, 512], BF)
            # ---- S1b: y-half of h1; r = y1 + u1 ----
            for m in range(4):
                pt = ps.tile([128, 512], F32, tag="ps")
                for kp in range(2):
                    nc.tensor.matmul(
                        pt[:], wlinY_sb[:, 2 * kp:2 * kp + 2, 128 * m:128 * (m + 1)],
                        x1t[:, 2 * kp:2 * kp + 2, :], perf_mode=DR,
                        start=(kp == 0), stop=(kp == 1))
                ytmp = scr.tile([128, 512], F8, tag="ytmp")
                nc.scalar.activation(ytmp[:], pt[:], AF.Relu,
                                     bias=blin_sb[:, m:m + 1])
                if KV_GPS:
                    nc.gpsimd.tensor_add(r[:, m, :], ytmp[:], u1[:, m, :])
                else:
                    nc.vector.tensor_add(r[:, m, :], ytmp[:], u1[:, m, :])

            # ---- S2: A = relu(W_down^T u1 + b_down); Bq likewise from u2;
            # paired PSUM banks, one act for both ----
            AB = ap_.tile([128, 2, 2, 512], F8)
            for m in range(2):
                pt12 = ps2.tile([128, 2, 512], F32, tag="ps2")
                for kp in range(2):
                    w = wdown_sb[:, 2 * kp:2 * kp + 2, 128 * m:128 * (m + 1)]
                    nc.tensor.matmul(pt12[:, 0, :], w, u1[:, 2 * kp:2 * kp + 2, :],
                                     perf_mode=DR,
                                     start=(kp == 0), stop=(kp == 1))
                    nc.tensor.matmul(pt12[:, 1, :], w, u2[:, 2 * kp:2 * kp + 2, :],
                                     perf_mode=DR,
                                     start=(kp == 0), stop=(kp == 1))
                nc.scalar.activation(AB[:, :, m, :], pt12[:], AF.Relu,
                                     bias=bdown_sb[:, m:m + 1])
            A = AB[:, 0]
            Bq = AB[:, 1]

            # ---- S3: q = Wq@Bq, k = Wk@A, v = Wv@A (channel-major);
            # q/k share a PSUM pair and a single copy act ----
            qk12 = qkvp.tile([128, 2, 2, 512], BF, tag="qk")
            v = qkvp.tile([128, 2, 512], BF, tag="v")
            for m in range(2):
                pt12 = ps2.tile([128, 2, 512], F32, tag="ps2")
                nc.tensor.matmul(pt12[:, 0, :], wq_sb[:, 0:2, 128 * m:128 * (m + 1)],
                                 Bq[:, 0:2, :], perf_mode=DR,
                                 start=True, stop=True)
                nc.tensor.matmul(pt12[:, 1, :], wk_sb[:, 0:2, 128 * m:128 * (m + 1)],
                                 A[:, 0:2, :], perf_mode=DR,
                                 start=True, stop=True)
                nc.scalar.activation(qk12[:, :, m, :], pt12[:], AF.Copy)
            for m in range(2):
                pt = ps.tile([128, 512], F32, tag="ps")
                nc.tensor.matmul(pt[:], wv_sb[:, 0:2, 128 * m:128 * (m + 1)],
                                 A[:, 0:2, :], perf_mode=DR,
                                 start=True, stop=True)
                nc.scalar.activation(v[:, m, :], pt[:], AF.Copy)

            return dict(x1b=x1b, r=r, A=A, q=qk12[:, 0], k=qk12[:, 1], v=v)

        def emit_back_head(j, st, segs=None):
            bbs = segs if segs is not None else _bbs(j)
            nb = len(bbs)
            col0 = bbs[0][1]
            colw = sum(s[2] for s in bbs)
            cs = slice(col0, col0 + colw)
            x1b, r, A = st["x1b"], st["r"], st["A"]
            q, k, v = st["q"], st["k"], st["v"]

            # ---- S4: per-branch L2 row norms; rn = exp(-0.5*ln(ssq)) ----
            # (rnq additionally folds temperature via its log as exp bias)
            for (name, t_) in (("q", q), ("k", k)):
                sq = sqp.tile([128, 2, 512], BF, tag="sq")
                nc.vector.tensor_mul(sq[:, :, cs], t_[:, :, cs], t_[:, :, cs])
                ssq = nrmp.tile([128, 2, nb], F32, tag="ssq" + name)
                w = colw // nb
                nc.vector.reduce_sum(
                    ssq[:], sq[:, :, cs].rearrange("p g (n w) -> p g n w", w=w),
                    axis=AX.X)
                rr = nrmp.tile([128, 2, nb], F32, tag="rn" + name)
                lnss = nrmp.tile([128, 2, nb], F32, tag="ln" + name)
                nc.scalar.activation(lnss[:], ssq[:], AF.Ln, bias=tiny_sb[:, 0:1])
                if name == "q":
                    for g in range(2):
                        nc.scalar.activation(rr[:, g, :], lnss[:, g, :], AF.Exp,
                                             scale=-0.5,
                                             bias=lntemp_sb[:, g:g + 1])
                else:
                    nc.scalar.activation(rr[:], lnss[:], AF.Exp, scale=-0.5)
                # scale q/k rows by their inverse norms, per branch segment
                for g in range(2):
                    for bi, (b, off, w_) in enumerate(bbs):
                        nc.vector.tensor_scalar_mul(
                            t_[:, g, off:off + w_], in0=t_[:, g, off:off + w_],
                            scalar1=rr[:, g, bi:bi + 1])

            # ---- S5: PE-transpose qn,kn -> token-major qT,kT ----
            tb0, tb1 = col0 // 128, (col0 + colw) // 128
            qT = qtp.tile([128, 4, 256], BF, tag="qT")
            kT = qtp.tile([128, 4, 256], BF, tag="kT")
            for (dst, src) in ((qT, q), (kT, k)):
                pt = atp.tile([128, 4, 256], BF, tag="atp")
                for tb in range(tb0, tb1):
                    for g in range(2):
                        nc.tensor.matmul(
                            pt[:, tb, 128 * g:128 * (g + 1)],
                            src[:, g, 128 * tb:128 * (tb + 1)], ident_sb[:],
                            is_transpose=True, start=(g == 0), stop=(g == 1))
                nc.vector.tensor_copy(dst[:, tb0:tb1, :], pt[:, tb0:tb1, :])

            # ---- S6: attention logits G = qn^T kn per (branch, head-group),
            # batched into shared PSUM tiles; exp via strided activations ----
            E = ep.tile([128, 2, 32 * nb], BF, tag="E")
            if j == 0:
                # stage kT into the pre-zeroed per-branch tiles
                for b in range(B_LOC):
                    rs = slice(64 * (b % 2), 64 * (b % 2) + 64)
                    nc.vector.tensor_copy(ktz[rs, b, :, :],
                                          kT[rs, b // 2, :].rearrange(
                                              "p (g c) -> p g c", c=128))
                # 4 waves of (g, half): each wave = 4 branches x 128 cols
                for g in range(2):
                    for h in range(2):
                        gz = atp.tile([128, 512], F32, tag="atp")
                        for bb in range(4):
                            b = 4 * h + bb
                            tb = b // 2
                            nc.tensor.matmul(
                                gz[:, 128 * bb:128 * (bb + 1)],
                                qT[:, tb, 128 * g:128 * (g + 1)],
                                ktz[:, b, g, :],
                                start=True, stop=True)
                        gzv = gz.rearrange("p (b c) -> p b c", c=128)
                        for pos in range(4):
                            if KV_EXPB:
                                nc.scalar.activation(
                                    E[32 * pos:32 * (pos + 1), g,
                                      32 * 4 * h:32 * 4 * (h + 1)].rearrange(
                                          "p (b c) -> p b c", c=32),
                                    gzv[32 * pos:32 * (pos + 1), :,
                                        32 * pos:32 * (pos + 1)],
                                    AF.Exp)
                            else:
                                for bb in range(4):
                                    bi = 4 * h + bb
                                    nc.scalar.activation(
                                        E[32 * pos:32 * (pos + 1), g,
                                          32 * bi:32 * (bi + 1)],
                                        gzv[32 * pos:32 * (pos + 1), bb,
                                            32 * pos:32 * (pos + 1)],
                                        AF.Exp)
            else:
                gx = atp.tile([128, 512], F32, tag="atp")
                for g in range(2):
                    for bi, (b, off, w_) in enumerate(bbs):
                        for ci in range(2):
                            tb = off // 128 + ci
                            nc.tensor.matmul(
                                gx[:, 256 * g + 128 * bi:256 * g + 128 * (bi + 1)],
                                qT[:, tb, 128 * g:128 * (g + 1)],
                                kT[:, tb, 128 * g:128 * (g + 1)],
                                start=(ci == 0), stop=(ci == 1))
                gxv = gx.rearrange("p (g b c) -> p g b c", g=2, c=128)
                for g in range(2):
                    for pos in range(4):
                        if KV_EXPB:
                            nc.scalar.activation(
                                E[32 * pos:32 * (pos + 1), g, :].rearrange(
                                    "p (b c) -> p b c", c=32),
                                gxv[32 * pos:32 * (pos + 1), g, 0:nb,
                                    32 * pos:32 * (pos + 1)],
                                AF.Exp)
                        else:
                            for bi in range(nb):
                                nc.scalar.activation(
                                    E[32 * pos:32 * (pos + 1), g,
                                      32 * bi:32 * (bi + 1)],
                                    gxv[32 * pos:32 * (pos + 1), g, bi,
                                        32 * pos:32 * (pos + 1)],
                                    AF.Exp)

            # ---- S7: softmax denominators + blockwise-transposed E ----
            S = nrmp.tile([128, 2, nb], F32, tag="S")
            R = nrmp.tile([128, 2, nb], F32, tag="R")
            nc.vector.reduce_sum(
                S[:], E[:].rearrange("p g (n w) -> p g n w", w=32), axis=AX.X)
            nc.vector.reciprocal(R[:], S[:])
            ET = ep.tile([128, 2, 32 * nb], BF, tag="ET")
            for g in range(2):
                nc.vector.transpose(ET[:, g, :], E[:, g, :])
            # block-diagonal stationary for AV: bd[32p:,g,bi,32p:] = ET block
            etv = ET.rearrange("p g (b c) -> p g b c", c=32)
            for g in range(2):
                for pos in range(4):
                    nc.vector.tensor_copy(
                        bd[32 * pos:32 * (pos + 1), g, 0:nb,
                           32 * pos:32 * (pos + 1)],
                        etv[32 * pos:32 * (pos + 1), g, :, :])

            # ---- S8: AV = (E^T blockdiag) @ v, then scale rows by 1/S ----
            av = avp.tile([128, 2, 512], F8)
            pav = [atp.tile([128, 512], F32, tag="atp", name=f"pav{g_}")
                   for g_ in range(2)]
            for bi, (b, off, w_) in enumerate(bbs):
                for g in range(2):
                    nc.tensor.matmul(
                        pav[g][:, off:off + w_], bd[:, g, bi, :],
                        v[:, g, off:off + w_], start=True, stop=True)
                for g in range(2):
                    nc.vector.tensor_scalar_mul(
                        av[:, g, off:off + w_], in0=pav[g][:, off:off + w_],
                        scalar1=R[:, g, bi:bi + 1])
            st["av", col0] = av

        def emit_back_tail(j, st, segs=None):
            bbs = segs if segs is not None else _bbs(j)
            col0 = bbs[0][1]
            colw = sum(s[2] for s in bbs)
            cs = slice(col0, col0 + colw)
            x1b, r, A = st["x1b"], st["r"], st["A"]
            av = st["av", col0]

            # ---- S9: o1 = Wo@av + A (res1) ----
            o1 = o1p.tile([128, 2, 512], F8)
            for m in range(2):
                pt = ps.tile([128, 512], F32, tag="ps")
                nc.tensor.matmul(pt[:, cs], wo_sb[:, 0:2, 128 * m:128 * (m + 1)],
                                 av[:, 0:2, cs], perf_mode=DR,
                                 start=True, stop=True)
                nc.vector.tensor_add(o1[:, m, cs], pt[:, cs], A[:, m, cs])

            # ---- S10: y = W_up^T o1 + b_up + r ----
            y = yp.tile([128, 4, 512], F8)
            for m in range(4):
                pt = ps.tile([128, 512], F32, tag="ps")
                nc.tensor.matmul(pt[:, cs], wup_sb[:, 0:2, 128 * m:128 * (m + 1)],
                                 o1[:, 0:2, cs], perf_mode=DR,
                                 start=True, stop=True)
                nc.vector.scalar_tensor_tensor(
                    y[:, m, cs], in0=pt[:, cs], scalar=bup_sb[:, m:m + 1],
                    in1=r[:, m, cs], op0=OP.add, op1=OP.add)

            # ---- S11: pre = W_end^T y + b_end + t1 ; LN stats via ones-matmul
            pre = prep.tile([128, 4, 512], BF)
            s1ps = atp.tile([128, 512], F32, tag="atp", name="s1ps")
            s2ps = atp.tile([128, 512], F32, tag="atp", name="s2ps")
            for m in range(4):
                pt = ps.tile([128, 512], F32, tag="ps")
                for kp in range(2):
                    nc.tensor.matmul(
                        pt[:, cs],
                        wend_sb[:, 2 * kp:2 * kp + 2, 128 * m:128 * (m + 1)],
                        y[:, 2 * kp:2 * kp + 2, cs], perf_mode=DR,
                        start=(kp == 0), stop=(kp == 1))
                nc.vector.scalar_tensor_tensor(
                    pre[:, m, cs], in0=pt[:, cs], scalar=bend_sb[:, m:m + 1],
                    in1=x1b[:, m, cs], op0=OP.add, op1=OP.add)
                p2 = scr.tile([128, 512], BF, tag="p2")
                nc.vector.tensor_mul(p2[:, cs], pre[:, m, cs], pre[:, m, cs])
                nc.tensor.matmul(s1ps[:, cs], ones_sb[:], pre[:, m, cs],
                                 start=(m == 0), stop=(m == 3))
                nc.tensor.matmul(s2ps[:, cs], ones_sb[:], p2[:, cs],
                                 start=(m == 0), stop=(m == 3))

            # ---- S12: mu/rstd (rows replicated); rstd = exp(-0.5 ln(var+eps))
            mu = statp.tile([128, 512], BF, tag="mu")
            nc.vector.tensor_scalar_mul(mu[:, cs], in0=s1ps[:, cs],
                                        scalar1=1.0 / DIM)
            m2 = statp.tile([128, 512], BF, tag="m2")
            nc.vector.tensor_mul(m2[:, cs], mu[:, cs], mu[:, cs])
            var = statp.tile([128, 512], F32, tag="var")
            nc.vector.scalar_tensor_tensor(var[:, cs], in0=s2ps[:, cs],
                                           scalar=1.0 / DIM,
                                           in1=m2[:, cs], op0=OP.mult,
                                           op1=OP.subtract)
            rstd = statp.tile([128, 512], BF, tag="rstd")
            lnv = statp.tile([128, 512], F32, tag="lnv")
            nc.scalar.activation(lnv[:, cs], var[:, cs], AF.Ln,
                                 bias=epsln_sb[:, 0:1])
            nc.scalar.activation(rstd[:, cs], lnv[:, cs], AF.Exp, scale=-0.5)

            # ---- S13: out = ((pre-mu)*rstd)*gamma + beta; store per m ----
            ot = outp.tile([128, 4, 512], BF)
            for m in range(4):
                t1 = scr.tile([128, 512], BF, tag="t1")
                nc.vector.tensor_sub(t1[:, cs], pre[:, m, cs], mu[:, cs])
                mgb = scr.tile([128, 512], BF, tag="mgb")
                nc.vector.tensor_mul(mgb[:, cs], t1[:, cs], rstd[:, cs])
                if KV_ID:
                    nc.scalar.activation(ot[:, m, cs], mgb[:, cs], AF.Identity,
                                         scale=gamma_sb[:, m:m + 1],
                                         bias=beta_sb[:, m:m + 1])
                else:
                    nc.vector.tensor_scalar(
                        ot[:, m, cs], in0=mgb[:, cs],
                        scalar1=gamma_sb[:, m:m + 1],
                        scalar2=beta_sb[:, m:m + 1], op0=OP.mult, op1=OP.add)
                nc.sync.dma_start(out_e[j, :, m, cs], ot[:, m, cs])

        prev = None
        order = [1, 2, 0, 3, 4]
        for j in order:
            first = j == order[0]
            def _crit_weights():
                nc.sync.dma_start(wall_sb[:, 0:2048], wall_e[:, 0:2048])
                nc.sync.dma_start(ball_sb[:], ball_e[:, :])

            ld = emit_loads(j, _crit_weights if first else None)
            if first:
                nc.sync.dma_start(wall_sb[:, 2048:4096], wall_e[:, 2048:4096])
                nc.sync.dma_start(wall_sb[:, 4096:7168], wall_e[:, 4096:7168])
                nc.sync.dma_start(wall_sb[:, 7168:10240], wall_e[:, 7168:10240])
                nc.sync.dma_start(ones_sb[:], ones_e[:, :])
                nc.sync.dma_start(ident_sb[:], ident_e[:, :])
                nc.sync.dma_start(bd[:], zeros_e[:, :, :, :])
                nc.sync.dma_start(ktz[:], zeros2_e[:, :, :, :])
                nc.scalar.activation(lntemp_sb[:], tempc_sb[:], AF.Ln)
            u12 = emit_front_S1u(j, ld)
            if prev is not None:
                emit_back_head(prev[0], prev[1])
            st = emit_front_rest(j, ld, u12)
            if prev is not None:
                emit_back_tail(prev[0], prev[1])
            prev = (j, st)
        # final tile: split the back into per-branch halves so the second
        # half's PE/scalar work overlaps the first half's LN tail
        jl, stl = prev
        fsegs = _bbs(jl)
        emit_back_head(jl, stl, [fsegs[0]])
        emit_back_head(jl, stl, [fsegs[1]])
        emit_back_tail(jl, stl, [fsegs[0]])
        emit_back_tail(jl, stl, [fsegs[1]])

    nc.compile()
    _elide_duplicate_ldweights(nc)
    return nc


def _elide_duplicate_ldweights(nc):
    """Drop an InstLdweights when the PE already holds those exact weights.

    S1/S2 issue back-to-back matmul pairs sharing a stationary operand
    (x1/x2 and u1/u2 against the same weight block); legalization still
    emits one InstLdweights per matmul.  The duplicate loads carry no
    semaphore waits/updates, so removing them only shortens the PE queue;
    the following matmul reuses the weights already resident in the array.
    """
    def ap_sig(ap):
        try:
            return (ap.memref, str(ap.ap), ap.offset)
        except Exception:
            return str(ap)

    for b in nc.main_func.blocks:
        keep = []
        last_ldw_sig = None
        last_was = None
        for i in b.instructions:
            drop = False
            if getattr(i, "engine", None) == mybir.EngineType.PE:
                if isinstance(i, mybir.InstLdweights):
                    sig = (ap_sig(i.ins[0]), str(i.perf_mode),
                           bool(i.is_transpose))
                    si = i.sync_info
                    clean = si is None or (not si.on_wait and not si.on_update)
                    if sig == last_ldw_sig and last_was == "mm" and clean:
                        drop = True
                    else:
                        last_ldw_sig = sig
                    last_was = "ldw"
                elif isinstance(i, mybir.InstMatmult):
                    last_was = "mm"
                else:
                    last_ldw_sig = None
                    last_was = None
            if not drop:
                keep.append(i)
        b.instructions = keep


# ---------------- host side ----------------
_CACHE = {}


def _get_runner():
    if "runner" in _CACHE:
        return _CACHE["runner"]
    import jax
    from jax.sharding import Mesh, PartitionSpec
    from jax.experimental.shard_map import shard_map
    from concourse.bass2jax import (
        _bass_exec_p, install_neuronx_cc_hook, partition_id_tensor)
    import concourse.mybir as mybir_

    nc = build_nc()
    install_neuronx_cc_hook()
    partition_name = nc.partition_id_tensor.name if nc.partition_id_tensor else None
    in_names, out_names, out_avals, zero_outs = [], [], [], []
    for alloc in nc.m.functions[0].allocations:
        if not isinstance(alloc, mybir_.MemoryLocationSet):
            continue
        name = alloc.memorylocations[0].name
        if alloc.kind == "ExternalInput":
            if name != partition_name:
                in_names.append(name)
        elif alloc.kind == "ExternalOutput":
            out_names.append(name)
            shape = tuple(alloc.tensor_shape)
            dtype = mybir_.dt.np(alloc.dtype)
            out_avals.append(jax.core.ShapedArray(shape, dtype))
            zero_outs.append(np.zeros(shape, dtype))
    n_params, n_outs = len(in_names), len(out_avals)
    all_in = list(in_names) + list(out_names)
    if partition_name is not None:
        all_in.append(partition_name)
    donate = tuple(range(n_params, n_params + n_outs))

    def _body(*args):
        operands = list(args)
        if partition_name is not None:
            operands.append(partition_id_tensor())
        return tuple(_bass_exec_p.bind(
            *operands, out_avals=tuple(out_avals), in_names=tuple(all_in),
            out_names=tuple(out_names), lowering_input_output_aliases=(),
            sim_require_finite=True, sim_require_nnan=True, nc=nc))

    devices = jax.devices()[:8]
    mesh = Mesh(np.asarray(devices), ("core",))
    fn = jax.jit(
        shard_map(_body, mesh=mesh,
                  in_specs=(PartitionSpec("core"),) * (n_params + n_outs),
                  out_specs=(PartitionSpec("core"),) * n_outs,
                  check_rep=False),
        donate_argnums=donate, keep_unused=True)
    _CACHE["runner"] = (fn, in_names, out_names, out_avals, zero_outs)
    return _CACHE["runner"]


def _pack_x(x, dt):
    """[64, 512, 320] f32 -> [8 cores, 5 tiles, 128, 4, 512] in dtype dt."""
    xall = np.asarray(x, np.float32).reshape(8, 8, 4, 128, NTOK)
    tiles = []
    # tile 0: all 8 local batches' 64 z-tokens -> col = b*64 + t
    t0 = xall[:, :, :, :, :64].transpose(0, 3, 2, 1, 4).reshape(8, 128, 4, 512)
    tiles.append(t0)
    # tiles 1..4: batches (2j-2, 2j-1), 256 x-tokens -> col = u*256 + (t-64)
    for j in range(1, 5):
        tj = xall[:, 2 * j - 2:2 * j, :, :, 64:].transpose(0, 3, 2, 1, 4)
        tiles.append(tj.reshape(8, 128, 4, 512))
    packed = np.stack(tiles, axis=1)  # [8, 5, 128, 4, 512]
    return np.ascontiguousarray(packed.astype(dt))


def _unpack_out(arr):
    """[8 cores, 5, 128, 4, 512] -> [64, 512, 320] f32."""
    arr = np.asarray(arr, np.float32)
    full = np.empty((8, 8, 512, NTOK), np.float32)
    t0 = arr[:, 0].reshape(8, 128, 4, 8, 64).transpose(0, 3, 2, 1, 4)
    full[:, :, :, :64] = t0.reshape(8, 8, 512, 64)
    for j in range(1, 5):
        tj = arr[:, j].reshape(8, 128, 4, 2, 256).transpose(0, 3, 2, 1, 4)
        full[:, 2 * j - 2:2 * j, :, 64:] = tj.reshape(8, 2, 512, 256)
    return full.reshape(64, 512, NTOK)


def _prep_inputs(inputs):
    import ml_dtypes
    BF_NP = ml_dtypes.bfloat16
    F8_NP = ml_dtypes.float8_e4m3
    f = lambda a: np.ascontiguousarray(np.asarray(a), dtype=np.float32)

    def wpack(w, kt):
        """[K, M] -> [128, kt*M] with row k = kt_idx*128 + p."""
        w = np.asarray(w, np.float32)
        K, M = w.shape
        return w.reshape(kt, 128, M).transpose(1, 0, 2).reshape(128, kt * M)

    x1f = np.asarray(inputs["x1"], np.float32).reshape(64, DIM, NTOK)
    x2f = np.asarray(inputs["x2"], np.float32).reshape(64, DIM, NTOK)
    x1_8 = _pack_x(x1f, F8_NP)
    x2_8 = _pack_x(x2f, F8_NP)
    x1_b = _pack_x(x1f, BF_NP)

    W_lin = np.asarray(inputs["W_lin"], np.float32)
    wall = np.concatenate([
        wpack(W_lin[:, DIM:], 4),                      # u-half
        wpack(W_lin[:, :DIM], 4),                      # y-half
        wpack(np.asarray(inputs["W_down"], np.float32), 4),
        wpack(np.asarray(inputs["Wq"], np.float32).T, 2),
        wpack(np.asarray(inputs["Wk"], np.float32).T, 2),
        wpack(np.asarray(inputs["Wv"], np.float32).T, 2),
        wpack(np.asarray(inputs["Wo"], np.float32).T, 2),
        wpack(np.asarray(inputs["W_up"], np.float32), 2),
        wpack(np.asarray(inputs["W_end"], np.float32), 4),
    ], axis=1).astype(F8_NP)
    assert wall.shape == (128, W_COLS)

    temp = f(inputs["temperature"]).reshape(HEADS)
    temp_col = np.empty((128, 2), np.float32)
    for g in range(2):
        for hh in range(4):
            temp_col[32 * hh:32 * (hh + 1), g] = temp[4 * g + hh]
    ball = np.concatenate([
        f(inputs["b_lin"]).reshape(8, 128).T,
        f(inputs["b_down"]).reshape(2, 128).T,
        f(inputs["b_up"]).reshape(4, 128).T,
        f(inputs["b_end"]).reshape(4, 128).T,
        f(inputs["gamma"]).reshape(4, 128).T,
        f(inputs["beta"]).reshape(4, 128).T,
        temp_col,
    ], axis=1)
    assert ball.shape == (128, 28)
    ball = np.ascontiguousarray(ball)

    shared = {
        "wall8": np.ascontiguousarray(wall), "ball": ball,
        "ones_in": np.ones((128, 128), BF_NP),
        "ident_in": np.eye(128, dtype=np.float32).astype(BF_NP),
        "zeros_in": np.zeros((128, 2, 8, 128), BF_NP),
        "zeros2_in": np.zeros((128, 8, 2, 128), BF_NP),
    }
    in_maps = []
    for c in range(8):
        m = dict(shared)
        m["x1f8"] = np.ascontiguousarray(x1_8[c])
        m["x2f8"] = np.ascontiguousarray(x2_8[c])
        m["x1bf"] = np.ascontiguousarray(x1_b[c])
        in_maps.append(m)
    return in_maps


def run_in_maps(in_maps):
    """Run the prebuilt executable on 8 cores; returns per-core out arrays."""
    import jax
    fn, in_names, out_names, out_avals, zero_outs = _get_runner()
    per_core = [[np.asarray(m[name]) for name in in_names] for m in in_maps]
    concat_in = [np.concatenate([per_core[c][i] for c in range(8)], axis=0)
                 for i in range(len(in_names))]
    concat_zeros = [np.zeros((8 * z.shape[0], *z.shape[1:]), z.dtype)
                    for z in zero_outs]
    out = fn(*concat_in, *concat_zeros)
    jax.block_until_ready(out)
    oi = out_names.index("out")
    arr = np.asarray(out[oi]).reshape(8, *out_avals[oi].shape)
    return arr


def kernel(**inputs):
    in_maps = _prep_inputs(inputs)
    arr = run_in_maps(in_maps)  # [8, 5, 128, 4, 512] bf16
    full = _unpack_out(arr).reshape(64, DIM, 16, 20)
    return full.astype(np.float32)


if __name__ == "__main__":
    rng = np.random.default_rng(0)
    ins = {
        "x1": rng.standard_normal((64, 512, 16, 20), dtype=np.float32),
        "x2": rng.standard_normal((64, 512, 16, 20), dtype=np.float32),
    }
    s = 0.02
    for nm, shape in [("W_lin", (512, 1024)), ("W_down", (512, 256)),
                      ("W_up", (256, 512)), ("Wq", (256, 256)),
                      ("Wk", (256, 256)), ("Wv", (256, 256)),
                      ("Wo", (256, 256)), ("W_end", (512, 512))]:
        ins[nm] = (rng.standard_normal(shape) * s).astype(np.float32)
    for nm, n in [("b_lin", 1024), ("b_down", 256), ("b_up", 512),
                  ("b_end", 512)]:
        ins[nm] = np.zeros(n, np.float32)
    ins["gamma"] = np.ones(512, np.float32)
    ins["beta"] = np.zeros(512, np.float32)
    ins["temperature"] = np.ones((8, 1, 1), np.float32)
    out = kernel(**ins)
    print("kernel ran, out shape", out.shape, "mean", float(np.abs(out).mean()))
